# revision 1
# baseline (speedup 1.0000x reference)
"""Trainium2 Bass kernel for nn_BiologicalMemory (retrieval_knn).

Computes, for B=256 queries against N=50000 stored memories (D=1024):
  cosine similarity -> argmax -> threshold 0.6 -> decode winner with Linear(D,D).

Sharding: memories split across 8 NeuronCores on N (6250 rows each, padded to
6272 = 49*128). Each core computes its local sims + argmax + decodes its local
candidate; the host picks the global winner per query (gather/unshard step).

On-device pipeline per core (engines balanced against the ~30 MB DMA floor):
  DMA   : stream memory tiles [128,1024] f32
  ACT   : fused square+accumulate -> row norms; sqrt; psum->sbuf sims copies
  GPSIMD: normalize_recip (divide rows by norm, cast to bf16)
  PE    : 128x128 transposes of normalized bf16 tiles; sims matmul (bf16,
          f32 accum); decode matmul
  DVE   : psum->sbuf transpose copies, sims evac, pairwise max tree,
          hardware max_index (argmax), masking
"""

import sys

if "/opt/trn_rl_repo" not in sys.path:
    sys.path.insert(0, "/opt/trn_rl_repo")

import numpy as np
import ml_dtypes

import concourse.bass as bass  # noqa: F401
import concourse.mybir as mybir
import concourse.tile as tile
from concourse import bacc, bass_utils
from concourse.bass import IndirectOffsetOnAxis
from concourse.masks import make_identity

FP32 = mybir.dt.float32
BF16 = mybir.dt.bfloat16
U32 = mybir.dt.uint32
AF = mybir.ActivationFunctionType
ALU = mybir.AluOpType
AX = mybir.AxisListType

B = 256      # queries
D = 1024     # embedding dim
N = 50000    # memories
O = 1024     # decoder output dim
NCORES = 8
NSH = N // NCORES              # 6250 memories per core
NT = (NSH + 127) // 128        # 49 tiles of 128 rows
NPAD = NT * 128                # 6272
THRESH = 0.6

# engine-balance knobs
NORM_DVE_EVERY = 3   # every k-th tile's sum-of-squares runs on DVE instead of ACT
SIMS_DVE_EVERY = 2   # every k-th sims chunk evacuates on DVE instead of ACT
NORM_ENGINE_PATTERN = "GDGA"  # normalize engine per tile: G=gpsimd, D=dve, A=act
DMA_PER_TILE = True           # per-tile DMAs overlap better across HWDGE queues

# stage toggles (bisection probes)
USE_GPSIMD_NORM = True
DO_NORMS = True
DO_TRANSPOSE = True
DO_MATMUL = True
DO_FINALE = True


def _stream_rep(tc, nc, pools, aps, dims):
    (pp, mp, mbp, sp, trp, scp, mtp, ptrp, pmmp) = pools
    (q_d, mem_d, wt_d, bias_d, dec_d, val_d) = aps
    (npad, b, d, o) = dims
    nt = npad // 128
    nbt = b // 128
    ndc = d // 128

    # ---- constants ----
    ident = pp.tile([128, 128], BF16, tag="ident")
    make_identity(nc, ident[:])
    ones_col = pp.tile([1, 128], BF16, tag="ones")
    nc.vector.memset(ones_col[:], 1.0)
    eps_col = pp.tile([128, 1], FP32, tag="eps")
    nc.vector.memset(eps_col[:], 1e-12)

    # ---- queries ----
    qt_sb = pp.tile([128, ndc * b], BF16, tag="qt")
    rqn = []
    for bt in range(nbt):
        qf = mp.tile([128, d], FP32, tag="qm", bufs=1)
        nc.sync.dma_start(out=qf[:], in_=q_d[bt * 128:(bt + 1) * 128, :])
        qsc = scp.tile([128, d], FP32, tag="qnsq", bufs=1)
        qn2 = sp.tile([128, 1], FP32, tag=f"qn2_{bt}")
        nc.scalar.activation(out=qsc[:], in_=qf[:], func=AF.Square,
                             accum_out=qn2[:])
        qn = sp.tile([128, 1], FP32, tag=f"qn_{bt}")
        nc.scalar.activation(out=qn[:], in_=qn2[:], func=AF.Sqrt, bias=eps_col[:])
        r = pp.tile([128, 1], FP32, tag=f"rqn{bt}")
        nc.vector.reciprocal(out=r[:], in_=qn[:])
        rqn.append(r)

        qb = mbp.tile([128, d], BF16, tag="qmb", bufs=1)
        nc.vector.tensor_copy(out=qb[:], in_=qf[:])
        pt = ptrp.tile([128, d], BF16, tag="ptr")
        for j in range(ndc):
            nc.tensor.transpose(pt[:, j * 128:(j + 1) * 128],
                                qb[:, j * 128:(j + 1) * 128], ident[:])
        nc.vector.tensor_copy(
            out=qt_sb[:].rearrange("p (j w) -> p j w", j=ndc)[:, :, bt * 128:(bt + 1) * 128],
            in_=pt[:].rearrange("p (j w) -> p j w", j=ndc),
        )

    sims = [pp.tile([128, npad], BF16, tag=f"sims{bt}", name=f"sims{bt}")
            for bt in range(nbt)]
    ngrp = (nt + 3) // 4
    cms = [pp.tile([128, ngrp], FP32, tag=f"cms{bt}", name=f"cms{bt}")
           for bt in range(nbt)]

    # ---- stream memory tiles (groups of 4 tiles = 512 rows) ----
    for g0 in range(0, nt, 4):
        gtiles = list(range(g0, min(g0 + 4, nt)))
        u = len(gtiles)
        w = 128 * u
        mt = mtp.tile([128, ndc * w], BF16, tag="mt")

        m_g = mp.tile([128, u * d], FP32, tag="m")
        if DMA_PER_TILE:
            for s2, t2 in enumerate(gtiles):
                nc.sync.dma_start(
                    out=m_g[:, s2 * d:(s2 + 1) * d],
                    in_=mem_d[t2 * 128:(t2 + 1) * 128, :])
        else:
            nc.sync.dma_start(
                out=m_g[:].rearrange("p (u k) -> p u k", u=u),
                in_=mem_d[g0 * 128: g0 * 128 + u * 128, :].rearrange(
                    "(u p) k -> p u k", p=128),
            )

        n2g = sp.tile([128, u], FP32, tag="n2g")
        for s, t in enumerate(gtiles):
            m = m_g[:, s * d:(s + 1) * d]
            if not DO_NORMS:
                nc.vector.memset(n2g[:, s:s + 1], 1024.0)
            elif t % NORM_DVE_EVERY == NORM_DVE_EVERY - 1:
                nsc = scp.tile([128, d], BF16, tag="nsq")
                nc.vector.scalar_tensor_tensor(
                    out=nsc[:], in0=m, scalar=0.0, in1=m,
                    op0=ALU.add, op1=ALU.mult, accum_out=n2g[:, s:s + 1])
            else:
                nsc = scp.tile([128, d], FP32, tag="nsq")
                nc.scalar.activation(out=nsc[:], in_=m, func=AF.Square,
                                     accum_out=n2g[:, s:s + 1])
        mng = sp.tile([128, u], FP32, tag="mng")
        for sq0 in range(0, u, 2):
            sqr = min(2, u - sq0)
            nc.scalar.activation(out=mng[:, sq0:sq0 + sqr],
                                 in_=n2g[:, sq0:sq0 + sqr],
                                 func=AF.Sqrt, bias=eps_col[:])

        mb_g = mbp.tile([128, u * d], BF16, tag="mb")
        for s, t in enumerate(gtiles):
            m = m_g[:, s * d:(s + 1) * d]
            mb = mb_g[:, s * d:(s + 1) * d]
            eng = NORM_ENGINE_PATTERN[t % len(NORM_ENGINE_PATTERN)] \
                if USE_GPSIMD_NORM else "D"
            if eng == "G":
                nc.gpsimd.normalize_recip(out_ap=mb, in_ap=m,
                                          denom_ap=mng[:, s:s + 1])
            else:
                mnr = sp.tile([128, 1], FP32, tag="mnr")
                nc.vector.reciprocal(out=mnr[:], in_=mng[:, s:s + 1])
                if eng == "A":
                    nc.scalar.activation(out=mb, in_=m, func=AF.Copy,
                                         scale=mnr[:])
                else:
                    nc.vector.tensor_scalar(out=mb, in0=m, scalar1=mnr[:],
                                            scalar2=None, op0=ALU.mult)

        if DO_TRANSPOSE:
            for s0 in range(0, u, 2):
                pr = min(2, u - s0)       # tiles in this psum pair
                pt = ptrp.tile([128, pr * d], BF16, tag="ptr")
                for v in range(pr):
                    for j in range(ndc):
                        nc.tensor.transpose(
                            pt[:, v * d + j * 128: v * d + (j + 1) * 128],
                            mb_g[:, (s0 + v) * d + j * 128:
                                 (s0 + v) * d + (j + 1) * 128],
                            ident[:])
                nc.vector.tensor_copy(
                    out=mt[:].rearrange("p (j t k) -> p j t k", j=ndc, k=128)
                        [:, :, s0:s0 + pr, :],
                    in_=pt[:].rearrange("p (t j k) -> p j t k", j=ndc, k=128),
                )
        else:
            nc.vector.tensor_copy(out=mt[:, 0:u * 128], in_=mb_g[:, 0:u * 128])

        if not DO_MATMUL:
            continue
        for bt in range(nbt):
            pd = pmmp.tile([128, w], FP32, tag="pdot")
            for j in range(ndc):
                nc.tensor.matmul(
                    pd[:],
                    lhsT=qt_sb[:, j * b + bt * 128: j * b + bt * 128 + 128],
                    rhs=mt[:, j * w:(j + 1) * w],
                    start=(j == 0), stop=(j == ndc - 1),
                )
            dst = sims[bt][:, g0 * 128: g0 * 128 + w]
            if (g0 // 4) % SIMS_DVE_EVERY == 0:
                nc.vector.tensor_scalar(out=dst, in0=pd[:], scalar1=rqn[bt][:],
                                        scalar2=None, op0=ALU.mult)
            else:
                nc.scalar.activation(out=dst, in_=pd[:], func=AF.Copy,
                                     scale=rqn[bt][:])
            nc.vector.tensor_reduce(out=cms[bt][:, g0 // 4:g0 // 4 + 1],
                                    in_=dst, axis=AX.X, op=ALU.max)

    # ---- finale ----
    if not DO_FINALE:
        for bt in range(nbt):
            gz = sp.tile([128, 1], FP32, tag=f"gz{bt}")
            nc.vector.memset(gz[:], 0.0)
            nc.sync.dma_start(out=val_d[bt:bt + 1, :], in_=gz[:])
            oz = pp.tile([128, o], FP32, tag=f"odec{bt}")
            nc.vector.memset(oz[:], 0.0)
            nc.sync.dma_start(out=dec_d[bt * 128:(bt + 1) * 128, :], in_=oz[:])
        return

    wt_sb = pp.tile([128, ndc * o], BF16, tag="wt")
    nc.sync.dma_start(
        out=wt_sb[:].rearrange("p (c f) -> p c f", c=ndc),
        in_=wt_d.rearrange("(c p) f -> p c f", p=128),
    )
    bias_f = pp.tile([1, o], FP32, tag="biasf")
    nc.sync.dma_start(out=bias_f[:], in_=bias_d[:])
    bias_bf = pp.tile([1, o], BF16, tag="biasbf")
    nc.vector.tensor_copy(out=bias_bf[:], in_=bias_f[:])

    xt_sb = pp.tile([128, ndc * b], BF16, tag="xt")
    masks = []
    for bt in range(nbt):
        gmaxf = sp.tile([128, 1], FP32, tag=f"gmaxf{bt}")
        nc.vector.tensor_reduce(out=gmaxf[:], in_=cms[bt][:], axis=AX.X,
                                op=ALU.max)
        gmaxb = sp.tile([128, 1], BF16, tag=f"gmaxb{bt}")
        nc.vector.tensor_copy(out=gmaxb[:], in_=gmaxf[:])
        nc.sync.dma_start(out=val_d[bt:bt + 1, :], in_=gmaxf[:])

        mask = pp.tile([128, 1], FP32, tag=f"mask{bt}")
        nc.vector.tensor_scalar(out=mask[:], in0=gmaxf[:], scalar1=THRESH,
                                scalar2=None, op0=ALU.is_gt)
        masks.append(mask)

        gmax8 = sp.tile([128, 8], BF16, tag=f"gmax8{bt}")
        nc.vector.tensor_copy(out=gmax8[:], in_=gmaxb[:].to_broadcast([128, 8]))
        idx8 = sp.tile([128, 8], U32, tag=f"idx8{bt}")
        nc.vector.max_index(out=idx8[:], in_max=gmax8[:], in_values=sims[bt][:])

        xg = mp.tile([128, d], FP32, tag="m")
        nc.gpsimd.indirect_dma_start(
            out=xg[:], out_offset=None, in_=mem_d[:],
            in_offset=IndirectOffsetOnAxis(ap=idx8[:, 0:1], axis=0),
        )
        xb = mbp.tile([128, d], BF16, tag="mb")
        nc.vector.tensor_copy(out=xb[:], in_=xg[:])
        pt = ptrp.tile([128, d], BF16, tag="ptr")
        for j in range(ndc):
            nc.tensor.transpose(pt[:, j * 128:(j + 1) * 128],
                                xb[:, j * 128:(j + 1) * 128], ident[:])
        nc.vector.tensor_copy(
            out=xt_sb[:].rearrange("p (j w) -> p j w", j=ndc)[:, :, bt * 128:(bt + 1) * 128],
            in_=pt[:].rearrange("p (j w) -> p j w", j=ndc),
        )

    for bt in range(nbt):
        odec = pp.tile([128, o], FP32, tag=f"odec{bt}")
        for oc in range(o // 512):
            pdec = pmmp.tile([128, 512], FP32, tag="pdot")
            for j in range(ndc):
                nc.tensor.matmul(
                    pdec[:],
                    lhsT=xt_sb[:, j * b + bt * 128: j * b + bt * 128 + 128],
                    rhs=wt_sb[:, j * o + oc * 512: j * o + (oc + 1) * 512],
                    start=(j == 0), stop=False,
                )
            nc.tensor.matmul(pdec[:], lhsT=ones_col[:],
                             rhs=bias_bf[:, oc * 512:(oc + 1) * 512],
                             start=False, stop=True)
            nc.vector.tensor_scalar(out=odec[:, oc * 512:(oc + 1) * 512],
                                    in0=pdec[:], scalar1=masks[bt][:],
                                    scalar2=None, op0=ALU.mult)
        nc.sync.dma_start(out=dec_d[bt * 128:(bt + 1) * 128, :], in_=odec[:])


def _build_body(tc, nc, q_d, mem_d, wt_d, bias_d, dec_d, val_d, npad, b, d, o,
                reps=1):
    with (
        tc.tile_pool(name="persist", bufs=1) as pp,
        tc.tile_pool(name="mload", bufs=4) as mp,
        tc.tile_pool(name="mbuf", bufs=3) as mbp,
        tc.tile_pool(name="small", bufs=4) as sp,
        tc.tile_pool(name="tree", bufs=1) as trp,
        tc.tile_pool(name="scratch", bufs=2) as scp,
        tc.tile_pool(name="mt", bufs=2) as mtp,
        tc.tile_pool(name="ptr", bufs=2, space="PSUM") as ptrp,
        tc.tile_pool(name="pmm", bufs=4, space="PSUM") as pmmp,
    ):
        pools = (pp, mp, mbp, sp, trp, scp, mtp, ptrp, pmmp)
        aps = (q_d, mem_d, wt_d, bias_d, dec_d, val_d)
        dims = (npad, b, d, o)
        for _rep in range(reps):
            _stream_rep(tc, nc, pools, aps, dims)


def build_kernel(npad=NPAD, b=B, d=D, o=O, reps=1):
    nc = bacc.Bacc("TRN2", target_bir_lowering=False, debug=False,
                   enable_asserts=False)
    q_d = nc.dram_tensor("q", [b, d], FP32, kind="ExternalInput").ap()
    mem_d = nc.dram_tensor("mem", [npad, d], FP32, kind="ExternalInput").ap()
    wt_d = nc.dram_tensor("wt", [d, o], BF16, kind="ExternalInput").ap()
    bias_d = nc.dram_tensor("bias", [1, o], FP32, kind="ExternalInput").ap()
    dec_d = nc.dram_tensor("dec", [b, o], FP32, kind="ExternalOutput").ap()
    val_d = nc.dram_tensor("val", [b // 128, 128], FP32, kind="ExternalOutput").ap()

    with tile.TileContext(nc) as tc:
        _build_body(tc, nc, q_d, mem_d, wt_d, bias_d, dec_d, val_d, npad, b, d, o,
                    reps=reps)
    nc.compile()
    return nc


_NC_CACHE = {}


def _get_nc():
    if "nc" not in _NC_CACHE:
        _NC_CACHE["nc"] = build_kernel()
    return _NC_CACHE["nc"]


def make_in_maps(query, memories, dec_w, dec_b):
    q = np.ascontiguousarray(np.asarray(query, dtype=np.float32))
    wt = np.ascontiguousarray(np.asarray(dec_w, dtype=np.float32).T).astype(
        ml_dtypes.bfloat16)
    bias = np.ascontiguousarray(np.asarray(dec_b, dtype=np.float32)).reshape(1, O)
    memories = np.asarray(memories, dtype=np.float32)
    in_maps = []
    for c in range(NCORES):
        sh = np.zeros((NPAD, D), np.float32)
        sh[:NSH] = memories[c * NSH:(c + 1) * NSH]
        in_maps.append({"q": q, "mem": sh, "wt": wt, "bias": bias})
    return in_maps


def combine_outputs(results):
    decs = np.stack([np.asarray(r["dec"]) for r in results])
    vals = np.stack([np.asarray(r["val"]).reshape(B) for r in results])
    win = np.argmax(vals, axis=0)
    return decs[win, np.arange(B)].astype(np.float32)


def run(query, memories, dec_w, dec_b, trace=False, **spmd_kwargs):
    nc = _get_nc()
    in_maps = make_in_maps(query, memories, dec_w, dec_b)
    res = bass_utils.run_bass_kernel_spmd(
        nc, in_maps, core_ids=list(range(NCORES)), trace=trace, **spmd_kwargs)
    return combine_outputs(res.results), res


def kernel(query, memories, dec_w, dec_b):
    out, _ = run(query, memories, dec_w, dec_b, trace=False)
    return out

